# revision 1
# baseline (speedup 1.0000x reference)
"""Multi-head attention TRN2 kernel, 8-core tensor-parallel (2 heads/core).

Strategy (per core c, head-slice cs = 128c:128c+128 of the projection dim):
  - Host passes X^T [1024, 8192] plus per-core weight slices, pre-transposed
    so every matmul operand lands in SBUF in its natural layout.
  - Projections compute Q^T/K^T/V^T [128, qlen] per batch (c-dim on
    partitions) via PE accumulation over 8 f-tiles; bias added on DVE.
  - V^T is PE-transposed into V_aug [j, d|ones] per head; the ones columns
    make the PV matmul also emit the softmax denominator D on partitions
    64:128 for free.
  - Scores are computed transposed (S^T = K^T.T @ Q^T tiles, j on
    partitions); softmax skips max-subtraction (scores are O(6) for this
    problem's distribution so exp cannot overflow); the attention mask is
    folded into the exp activation as a per-partition bias (0 or -1e30).
  - PV: C_un^T = V_aug.T @ exp(S^T), normalization C = C_un * recip(D) on
    DVE, out-proj back to natural [t, D] layout, partials summed on host.
  - All matmuls run as float32r (1 cycle/row at N>=512, ~1e-4 rel err).
  - Software pipelining: the attention phase is ACT(exp)-bound, so the next
    batch's X-DMA/projections/V-transposes and the previous batch's
    out-projection are interleaved into it through a dedicated 2-bank PSUM
    slot, keeping PE busy during the exp waits.
"""

import sys
from collections import deque

sys.path.insert(0, "/opt/trn_rl_repo")

import numpy as np

BS, QLEN, DIM, NH = 4, 2048, 1024, 16
DH = DIM // NH  # 64
NCORES = 8
CPD = DIM // NCORES  # 128 projection dims per core = 2 heads
T_FULL = BS * QLEN
NEG_BIAS = -1.0e30


def build_nc(bs=BS, qlen=QLEN, repeat=1):
    """Build + compile the per-core Bass program (same program on all cores).

    repeat>1 re-runs the whole computation (for timing amplification only).
    """
    import concourse.bass as bass_mod
    import concourse.mybir as mybir
    import concourse.tile as tile
    from concourse import bacc
    from concourse import masks
    from contextlib import ExitStack

    f32 = mybir.dt.float32
    f32r = mybir.dt.float32r
    EXP = mybir.ActivationFunctionType.Exp
    MULT = mybir.AluOpType.mult

    assert qlen % 512 == 0
    t_total = bs * qlen
    n_f = DIM // 128  # 8 f-tiles in the contraction over DIM
    jt = qlen // 128  # j-tiles (k-positions) per batch
    IC = min(1024, qlen)  # i-chunk width per head
    n_ic = qlen // IC
    NSL = IC // 512  # 512-slices per i-chunk
    tsl = qlen // 512  # 512-slices per batch for projections

    nc = bacc.Bacc()
    xt = nc.declare_dram_parameter("xt", [DIM, t_total], f32r, isOutput=False)
    wq = nc.declare_dram_parameter("wq", [DIM, CPD], f32r, isOutput=False)
    wk = nc.declare_dram_parameter("wk", [DIM, CPD], f32r, isOutput=False)
    wv = nc.declare_dram_parameter("wv", [DIM, CPD], f32r, isOutput=False)
    wo = nc.declare_dram_parameter("wo", [CPD, DIM], f32r, isOutput=False)
    bq = nc.declare_dram_parameter("bq", [CPD, 1], f32, isOutput=False)
    bk = nc.declare_dram_parameter("bk", [CPD, 1], f32, isOutput=False)
    bv = nc.declare_dram_parameter("bv", [CPD, 1], f32, isOutput=False)
    mb = nc.declare_dram_parameter("mb", [128, bs * jt], f32, isOutput=False)
    ones1 = nc.declare_dram_parameter("ones1", [128, 64], f32r, isOutput=False)
    out = nc.declare_dram_parameter("out", [t_total, DIM], f32, isOutput=True)

    xt_r = xt.rearrange("(n p) t -> n p t", p=128)

    with ExitStack() as ctx:
        tc = ctx.enter_context(tile.TileContext(nc))
        wpool = ctx.enter_context(tc.tile_pool(name="wpool", bufs=1))
        xpool = ctx.enter_context(tc.tile_pool(name="xpool", bufs=4))
        qkp = ctx.enter_context(tc.tile_pool(name="qkp", bufs=2))
        vtp = ctx.enter_context(tc.tile_pool(name="vtp", bufs=1))
        vap = ctx.enter_context(tc.tile_pool(name="vap", bufs=2))
        epool = ctx.enter_context(tc.tile_pool(name="epool", bufs=6))
        rpool = ctx.enter_context(tc.tile_pool(name="rpool", bufs=1))
        cpool = ctx.enter_context(tc.tile_pool(name="cpool", bufs=2))
        opool = ctx.enter_context(tc.tile_pool(name="opool", bufs=2))
        psS = ctx.enter_context(tc.tile_pool(name="psS", bufs=2, space="PSUM"))
        psC = ctx.enter_context(tc.tile_pool(name="psC", bufs=1, space="PSUM"))
        psX = ctx.enter_context(tc.tile_pool(name="psX", bufs=1, space="PSUM"))

        # ---- persistent weights / constants ----
        w_q = wpool.tile([128, n_f, CPD], f32r, tag="w_q")
        w_k = wpool.tile([128, n_f, CPD], f32r, tag="w_k")
        w_v = wpool.tile([128, n_f, CPD], f32r, tag="w_v")
        w_o = wpool.tile([128, DIM], f32r, tag="w_o")
        b_q = wpool.tile([128, 1], f32, tag="b_q")
        b_k = wpool.tile([128, 1], f32, tag="b_k")
        b_v = wpool.tile([128, 1], f32, tag="b_v")
        mb_s = wpool.tile([128, bs * jt], f32, tag="mb")
        ident = wpool.tile([128, 128], f32, tag="ident")

        # Order matters: these share the HWDGE queue with the first X-slice
        # DMA, so only what the first projection group needs goes first.
        nc.sync.dma_start(out=w_q[:], in_=wq.rearrange("(n p) m -> p n m", p=128))
        nc.sync.dma_start(out=b_q[:], in_=bq[:])

        def emit_late_consts():
            nc.sync.dma_start(
                out=w_k[:], in_=wk.rearrange("(n p) m -> p n m", p=128)
            )
            nc.sync.dma_start(out=b_k[:], in_=bk[:])
            nc.sync.dma_start(
                out=w_v[:], in_=wv.rearrange("(n p) m -> p n m", p=128)
            )
            nc.sync.dma_start(out=b_v[:], in_=bv[:])
            nc.sync.dma_start(out=w_o[:], in_=wo[:])
            nc.sync.dma_start(out=mb_s[:], in_=mb[:])
        masks.make_identity(nc, ident[:])

        ones_bcast = bass_mod.AP(
            tensor=ones1, offset=0, ap=[[64, 128], [0, jt], [1, 64]]
        )

        # per-batch tile sets, allocated one batch ahead
        tiles = {}

        def alloc_tiles(b):
            t = {}
            t["qT"] = qkp.tile([128, qlen], f32r, tag="qT", name=f"qT{b}")
            t["kT"] = qkp.tile([128, qlen], f32r, tag="kT", name=f"kT{b}")
            t["vT"] = vtp.tile([128, qlen], f32, tag="vT", name=f"vT{b}")
            t["vA"] = vap.tile([128, qlen], f32r, tag="vaugA", name=f"vA{b}")
            t["vB"] = vap.tile([128, qlen], f32r, tag="vaugB", name=f"vB{b}")
            t["ctx"] = cpool.tile([128, qlen], f32r, tag="ctx", name=f"ctx{b}")
            tiles[b] = t
            return t

        def emit_x_dma(vb, i):
            b = vb % bs
            xi = xpool.tile([128, n_f, 512], f32r, tag="x", name=f"x{vb}_{i}")
            src = xt_r[:, :, b * qlen + i * 512 : b * qlen + (i + 1) * 512]
            # The very first X slice rides the (startup-idle) ACT DGE queue so
            # it lands in parallel with the weight DMAs on the SP queue.
            eng = nc.scalar if (vb == 0 and i == 0) else nc.sync
            eng.dma_start(out=xi[:], in_=src.rearrange("f p t -> p f t"))
            return xi

        aux_n = [0]

        def next_aux(name):
            tg = ("auxA", "auxB")[aux_n[0] % 2]
            aux_n[0] += 1
            return psX.tile([128, 512], f32, tag=tg, name=f"{name}_{aux_n[0]}")

        def emit_proj(b, i, xi, w_s, b_s, dst):
            pp = next_aux(f"pp{b}_{i}")
            for f in range(n_f):
                nc.tensor.matmul(
                    pp[:],
                    w_s[:, f, :],
                    xi[:, f, :],
                    start=(f == 0),
                    stop=(f == n_f - 1),
                )
            nc.vector.tensor_scalar_add(
                dst[:, i * 512 : (i + 1) * 512], pp[:], b_s[:]
            )

        def emit_vaug_ones(b):
            for key in ("vA", "vB"):
                va = tiles[b][key]
                va_dst = va[:].rearrange("p (j c) -> p j c", c=128)[:, :, 64:128]
                nc.sync.dma_start(out=va_dst, in_=ones_bcast)

        def emit_vtrans(b, j):
            t = tiles[b]
            pt = next_aux(f"pt{b}_{j}")
            nc.tensor.transpose(
                pt[:, 0:128], t["vT"][:, j * 128 : (j + 1) * 128], ident[:]
            )
            nc.vector.tensor_copy(t["vA"][:, j * 128 : j * 128 + 64], pt[:, 0:64])
            nc.vector.tensor_copy(
                t["vB"][:, j * 128 : j * 128 + 64], pt[:, 64:128]
            )

        def emit_outproj(vb, t_idx):
            b = vb % bs
            t = tiles[vb]
            og = opool.tile([128, 1024], f32, tag="og", name=f"og{b}_{t_idx}")
            for dh in range(2):
                pO = next_aux(f"pO{vb}_{t_idx}_{dh}")
                nc.tensor.matmul(
                    pO[:],
                    t["ctx"][:, t_idx * 128 : (t_idx + 1) * 128],
                    w_o[:, dh * 512 : (dh + 1) * 512],
                    start=True,
                    stop=True,
                )
                nc.vector.tensor_copy(og[:, dh * 512 : (dh + 1) * 512], pO[:])
            nc.sync.dma_start(
                out=out[
                    b * qlen + t_idx * 128 : b * qlen + (t_idx + 1) * 128, :
                ],
                in_=og[:],
            )

        def proj_closures(b):
            """DMA + projection + V-transpose closures for batch b.

            X-slice DMAs lead their projection groups by two slices so the
            PE never waits on an in-flight DMA inside the aux stream.
            """
            t = tiles[b]
            cls = []
            xis = {}

            def dma_cl(i):
                def run():
                    xis[i] = emit_x_dma(b, i)

                return run

            for i in range(min(3, tsl)):
                cls.append(dma_cl(i))
            for i in range(tsl):
                if i + 3 < tsl:
                    cls.append(dma_cl(i + 3))
                for w_s, b_s, key in (
                    (w_q, b_q, "qT"),
                    (w_k, b_k, "kT"),
                    (w_v, b_v, "vT"),
                ):
                    cls.append(
                        lambda i=i, w_s=w_s, b_s=b_s, key=key: emit_proj(
                            b, i, xis[i], w_s, b_s, t[key]
                        )
                    )
            cls.append(lambda: emit_vaug_ones(b))
            for j in range(jt):
                cls.append(lambda j=j: emit_vtrans(b, j))
            return cls

        aux_q = deque()

        def pump(n=1):
            for _ in range(n):
                if not aux_q:
                    return
                aux_q.popleft()()

        # ---- startup: batch 0 front matter emitted directly ----
        n_vb = bs * repeat  # virtual batches (repeat only amplifies timing)
        alloc_tiles(0)
        cls0 = proj_closures(0)
        for cl in cls0[:2]:  # first two X-slice DMAs
            cl()
        emit_late_consts()
        for cl in cls0[2:]:
            cl()

        # Attention i-chunks are 512 wide; the score PSUM tile holds both
        # heads side by side ([A | B]) so one exp op covers both and the two
        # K=64 score matmuls land in disjoint PE row groups (concurrent).
        n_ica = qlen // 512
        for vb in range(n_vb):
            b = vb % bs
            t = tiles[vb]
            qT, kT, ctxt = t["qT"], t["kT"], t["ctx"]
            vaug = [t["vA"], t["vB"]]
            # stage next batch's front matter into the aux stream
            if vb + 1 < n_vb:
                alloc_tiles(vb + 1)
                for cl in proj_closures(vb + 1):
                    aux_q.append(cl)

            for ic in range(n_ica):
                isl = slice(ic * 512, (ic + 1) * 512)
                pCa = psC.tile([128, 512], f32, tag="pca", name=f"pCa{vb}_{ic}")
                pCb = psC.tile([128, 512], f32, tag="pcb", name=f"pCb{vb}_{ic}")
                pC = [pCa, pCb]
                # Software-pipelined: PV for j-1 is issued after scores/exp
                # for j, so the in-order PE stream never waits on the exp of
                # the current iteration before reaching the next scores.
                def emit_pv(jj, exx):
                    for h in range(2):
                        nc.tensor.matmul(
                            pC[h][:, 0:512],
                            vaug[h][:, jj * 128 : (jj + 1) * 128],
                            exx[:, h * 512 : (h + 1) * 512],
                            start=(jj == 0),
                            stop=(jj == jt - 1),
                        )

                pend = deque()  # (j, ex) whose PV is not yet emitted
                for j in range(jt):
                    pS = psS.tile([128, 1024], f32, tag="ps", name=f"pS{vb}_{ic}_{j}")
                    for h in range(2):
                        hp = slice(64 * h, 64 * h + 64)
                        nc.tensor.matmul(
                            pS[:, h * 512 : (h + 1) * 512],
                            kT[hp, j * 128 : (j + 1) * 128],
                            qT[hp, isl],
                            start=True,
                            stop=True,
                        )
                    ex = epool.tile([128, 1024], f32r, tag="e", name=f"e{vb}_{ic}_{j}")
                    nc.scalar.activation(
                        ex[:],
                        pS[:],
                        EXP,
                        bias=mb_s[:, b * jt + j : b * jt + j + 1],
                        scale=1.0,
                    )
                    pend.append((j, ex))
                    if len(pend) > 4:
                        emit_pv(*pend.popleft())
                    pump(1)
                while pend:
                    emit_pv(*pend.popleft())
                for h in range(2):
                    hp = slice(64 * h, 64 * h + 64)
                    rr = rpool.tile([128, 512], f32, tag="r", name=f"r{vb}_{ic}_{h}")
                    nc.vector.reciprocal(rr[0:64, :], pC[h][64:128, :])
                    nc.vector.tensor_tensor(
                        ctxt[hp, isl], pC[h][0:64, :], rr[0:64, :], op=MULT
                    )
                # ctx columns for this i-chunk are complete: queue out-proj
                for ti in range(ic * 4, (ic + 1) * 4):
                    aux_q.append(lambda ti=ti, bb=vb: emit_outproj(bb, ti))

        # drain remaining aux work (at least the last i-chunk's out-proj)
        pump(len(aux_q))

    nc.compile()
    return nc


_NC_CACHE = {}


def _get_nc(bs=BS, qlen=QLEN):
    key = (bs, qlen)
    if key not in _NC_CACHE:
        _NC_CACHE[key] = build_nc(bs, qlen)
    return _NC_CACHE[key]


def make_in_maps(hidden_states, attention_mask, Wq, bq, Wk, bk, Wv, bv, Wo, bo):
    """Host-side sharding: per-core input dicts."""
    bs, qlen, dim = hidden_states.shape
    x = np.ascontiguousarray(
        hidden_states.reshape(bs * qlen, dim).T, dtype=np.float32
    )
    scale = 1.0 / np.sqrt(np.float32(DH))
    jt = qlen // 128
    maskbias = np.where(attention_mask == 0, np.float32(NEG_BIAS), np.float32(0.0))
    # mb[p, b*jt + j] = maskbias[b, j*128 + p]
    mb = np.ascontiguousarray(
        maskbias.reshape(bs, jt, 128).transpose(2, 0, 1).reshape(128, bs * jt),
        dtype=np.float32,
    )
    in_maps = []
    for c in range(NCORES):
        cs = slice(c * CPD, (c + 1) * CPD)
        in_maps.append(
            {
                "xt": x,
                "wq": np.ascontiguousarray((Wq[cs] * scale).T, dtype=np.float32),
                "wk": np.ascontiguousarray(Wk[cs].T, dtype=np.float32),
                "wv": np.ascontiguousarray(Wv[cs].T, dtype=np.float32),
                "wo": np.ascontiguousarray(Wo[:, cs].T, dtype=np.float32),
                "bq": np.ascontiguousarray(
                    (bq[cs] * scale)[:, None], dtype=np.float32
                ),
                "bk": np.ascontiguousarray(bk[cs][:, None], dtype=np.float32),
                "bv": np.ascontiguousarray(bv[cs][:, None], dtype=np.float32),
                "mb": mb,
                "ones1": np.ones((128, 64), np.float32),
            }
        )
    return in_maps


def kernel(hidden_states, attention_mask, Wq, bq, Wk, bk, Wv, bv, Wo, bo):
    from concourse.bass_utils import run_bass_kernel_spmd

    hidden_states = np.asarray(hidden_states, dtype=np.float32)
    attention_mask = np.asarray(attention_mask)
    Wq, bq = np.asarray(Wq, np.float32), np.asarray(bq, np.float32)
    Wk, bk = np.asarray(Wk, np.float32), np.asarray(bk, np.float32)
    Wv, bv = np.asarray(Wv, np.float32), np.asarray(bv, np.float32)
    Wo, bo = np.asarray(Wo, np.float32), np.asarray(bo, np.float32)

    bs, qlen, dim = hidden_states.shape
    nc = _get_nc(bs, qlen)
    in_maps = make_in_maps(
        hidden_states, attention_mask, Wq, bq, Wk, bk, Wv, bv, Wo, bo
    )
    res = run_bass_kernel_spmd(nc, in_maps, list(range(NCORES)))
    acc = res.results[0]["out"].astype(np.float32)
    for c in range(1, NCORES):
        acc = acc + res.results[c]["out"]
    acc = acc + bo[None, :]
    return acc.reshape(bs, qlen, dim)



# revision 7
# speedup vs baseline: 1.0422x; 1.0422x over previous
"""Multi-head attention TRN2 kernel, 8-core tensor-parallel (2 heads/core).

Strategy (per core c, head-slice cs = 128c:128c+128 of the projection dim):
  - Host passes X^T [1024, 8192] plus per-core weight slices, pre-transposed
    so every matmul operand lands in SBUF in its natural layout.
  - Projections compute Q^T/K^T [128, qlen] f32r and V^T [128, qlen] bf16
    per batch via PE accumulation over 8 f-tiles; bias added on DVE.
  - V^T is PE-transposed (bf16 identity, 1 cyc/row) into per-head V tiles
    vh[h] [j, 65] per j-tile: 64 head dims plus a ones column so the PV
    matmul also emits the softmax denominator for free.
  - Scores are computed transposed (S^T = K^T.T @ Q^T tiles, j on
    partitions); softmax skips max-subtraction (scores are O(6) for this
    problem's distribution so exp cannot overflow); the attention mask is
    folded into the exp activation as a per-partition bias (0 or -1e30).
  - PV runs in the narrow orientation: ctx[i, d] (i on partitions) with
    bf16 operands, N=65 per matmul instead of N=512 — half the PE cycles
    of the transposed orientation. ctx is normalized on DVE (recip of the
    ones-column sums), PE-transposed back to [d, t] for the out-proj.
  - Out-proj in bf16 (lhsT = ctx^T, rhs = Wo slice), partials written
    bf16 and summed on host.
  - Aux work (next batch's X-DMA/projections/V-transposes, previous
    i-chunks' out-projections) is interleaved into the ACT(exp)-paced
    attention loop with a PE-cycle budget per j-iteration so the PE never
    idles (idle resets the p-state ramp) and never outruns the 2-deep
    score-PSUM ring.
"""

import sys
from collections import deque

sys.path.insert(0, "/opt/trn_rl_repo")

import numpy as np

BS, QLEN, DIM, NH = 4, 2048, 1024, 16
DH = DIM // NH  # 64
NCORES = 8
CPD = DIM // NCORES  # 128 projection dims per core = 2 heads
T_FULL = BS * QLEN
NEG_BIAS = -1.0e30


def build_nc(bs=BS, qlen=QLEN):
    """Build + compile the per-core Bass program (same program on all cores)."""
    import concourse.bass as bass_mod
    import concourse.mybir as mybir
    import concourse.tile as tile
    from concourse import bacc
    from concourse import masks
    from contextlib import ExitStack

    f32 = mybir.dt.float32
    f32r = mybir.dt.float32r
    bf16 = mybir.dt.bfloat16
    EXP = mybir.ActivationFunctionType.Exp

    assert qlen % 512 == 0
    t_total = bs * qlen
    n_f = DIM // 128  # 8 f-tiles in the contraction over DIM
    jt = qlen // 128  # j-tiles (k-positions) per batch
    tsl = qlen // 512  # 512-slices per batch for projections
    n_ica = qlen // 512  # i-chunks per batch

    nc = bacc.Bacc()
    xt = nc.declare_dram_parameter("xt", [DIM, t_total], f32r, isOutput=False)
    wq = nc.declare_dram_parameter("wq", [DIM, CPD], f32r, isOutput=False)
    wk = nc.declare_dram_parameter("wk", [DIM, CPD], f32r, isOutput=False)
    wv = nc.declare_dram_parameter("wv", [DIM, CPD], f32r, isOutput=False)
    wo = nc.declare_dram_parameter("wo", [CPD, DIM], bf16, isOutput=False)
    bq = nc.declare_dram_parameter("bq", [CPD, 1], f32, isOutput=False)
    bk = nc.declare_dram_parameter("bk", [CPD, 1], f32, isOutput=False)
    bv = nc.declare_dram_parameter("bv", [CPD, 1], f32, isOutput=False)
    mb = nc.declare_dram_parameter("mb", [128, bs * jt], f32, isOutput=False)
    out = nc.declare_dram_parameter("out", [t_total, DIM], bf16, isOutput=True)

    xt_r = xt.rearrange("(n p) t -> n p t", p=128)

    with ExitStack() as ctx:
        tc = ctx.enter_context(tile.TileContext(nc))
        wpool = ctx.enter_context(tc.tile_pool(name="wpool", bufs=1))
        xpool = ctx.enter_context(tc.tile_pool(name="xpool", bufs=4))
        qkp = ctx.enter_context(tc.tile_pool(name="qkp", bufs=2))
        vtp = ctx.enter_context(tc.tile_pool(name="vtp", bufs=1))
        vhp = ctx.enter_context(tc.tile_pool(name="vhp", bufs=2))
        epool = ctx.enter_context(tc.tile_pool(name="epool", bufs=6))
        cxp = ctx.enter_context(tc.tile_pool(name="cxp", bufs=2))
        rrp = ctx.enter_context(tc.tile_pool(name="rrp", bufs=2))
        ctp = ctx.enter_context(tc.tile_pool(name="ctp", bufs=2))
        opool = ctx.enter_context(tc.tile_pool(name="opool", bufs=2))
        psS = ctx.enter_context(tc.tile_pool(name="psS", bufs=2, space="PSUM"))
        psC = ctx.enter_context(tc.tile_pool(name="psC", bufs=1, space="PSUM"))
        psX = ctx.enter_context(tc.tile_pool(name="psX", bufs=1, space="PSUM"))

        # ---- persistent weights / constants ----
        w_q = wpool.tile([128, n_f, CPD], f32r, tag="w_q")
        w_k = wpool.tile([128, n_f, CPD], f32r, tag="w_k")
        w_v = wpool.tile([128, n_f, CPD], f32r, tag="w_v")
        w_o = wpool.tile([128, DIM], bf16, tag="w_o")
        b_q = wpool.tile([128, 1], f32, tag="b_q")
        b_k = wpool.tile([128, 1], f32, tag="b_k")
        b_v = wpool.tile([128, 1], f32, tag="b_v")
        mb_s = wpool.tile([128, bs * jt], f32, tag="mb")
        ident = wpool.tile([128, 128], bf16, tag="ident")

        # Order matters: these share the HWDGE queue with the first X-slice
        # DMA, so only what the first projection group needs goes first.
        nc.sync.dma_start(out=w_q[:], in_=wq.rearrange("(n p) m -> p n m", p=128))
        nc.sync.dma_start(out=b_q[:], in_=bq[:])

        def emit_late_consts():
            nc.sync.dma_start(
                out=w_k[:], in_=wk.rearrange("(n p) m -> p n m", p=128)
            )
            nc.sync.dma_start(out=b_k[:], in_=bk[:])
            nc.sync.dma_start(
                out=w_v[:], in_=wv.rearrange("(n p) m -> p n m", p=128)
            )
            nc.sync.dma_start(out=b_v[:], in_=bv[:])
            nc.sync.dma_start(out=w_o[:], in_=wo[:])
            nc.sync.dma_start(out=mb_s[:], in_=mb[:])
        masks.make_identity(nc, ident[:])

        # per-batch tile sets, allocated one batch ahead
        tiles = {}

        def alloc_tiles(b):
            t = {}
            t["qT"] = qkp.tile([128, qlen], f32r, tag="qT", name=f"qT{b}")
            t["kT"] = qkp.tile([128, qlen], f32r, tag="kT", name=f"kT{b}")
            t["vT"] = vtp.tile([128, qlen], bf16, tag="vT", name=f"vT{b}")
            t["vh"] = [
                vhp.tile([128, jt * 65], bf16, tag=f"vh{h}", name=f"vh{h}_{b}")
                for h in range(2)
            ]
            t["ctxT"] = ctp.tile([128, qlen], bf16, tag="ctxT", name=f"ctxT{b}")
            tiles[b] = t
            return t

        def emit_x_dma(b, i):
            xi = xpool.tile([128, n_f, 512], f32r, tag="x", name=f"x{b}_{i}")
            src = xt_r[:, :, b * qlen + i * 512 : b * qlen + (i + 1) * 512]
            # The very first X slice rides the (startup-idle) ACT DGE queue so
            # it lands in parallel with the weight DMAs on the SP queue.
            eng = nc.scalar if (b == 0 and i == 0) else nc.sync
            eng.dma_start(out=xi[:], in_=src.rearrange("f p t -> p f t"))
            return xi

        aux_n = [0]

        def next_aux(name):
            tg = ("auxA", "auxB")[aux_n[0] % 2]
            aux_n[0] += 1
            return psX.tile([128, 512], f32, tag=tg, name=f"{name}_{aux_n[0]}")

        def next_aux_bf(name):
            # Same psX slot rotation, viewed as bf16 (same byte size).
            tg = ("auxA", "auxB")[aux_n[0] % 2]
            aux_n[0] += 1
            return psX.tile([128, 1024], bf16, tag=tg, name=f"{name}_{aux_n[0]}")

        def emit_proj_half(b, i, xi, w_s, f0, pp):
            for f in range(f0, f0 + n_f // 2):
                nc.tensor.matmul(
                    pp[:],
                    w_s[:, f, :],
                    xi[:, f, :],
                    start=(f == 0),
                    stop=(f == n_f - 1),
                )

        def emit_bias(b, i, b_s, dst, pp):
            nc.vector.tensor_scalar_add(
                dst[:, i * 512 : (i + 1) * 512], pp[:], b_s[:]
            )

        def emit_vtrans(b, j):
            t = tiles[b]
            pt = next_aux_bf(f"pt{b}_{j}")
            nc.tensor.transpose(
                pt[:, 0:128], t["vT"][:, j * 128 : (j + 1) * 128], ident[:]
            )
            for h in range(2):
                nc.vector.tensor_copy(
                    t["vh"][h][:, j * 65 : j * 65 + 64],
                    pt[:, h * 64 : h * 64 + 64],
                )

        def emit_vh_ones(b):
            for h in range(2):
                vh_c = tiles[b]["vh"][h][:].rearrange(
                    "p (j c) -> p j c", c=65
                )[:, :, 64:65]
                nc.vector.memset(vh_c, 1.0)

        def emit_outproj_half(vb, t_idx, dh, og):
            b = vb % bs
            t = tiles[vb]
            pO = next_aux(f"pO{vb}_{t_idx}_{dh}")
            nc.tensor.matmul(
                pO[:],
                t["ctxT"][:, t_idx * 128 : (t_idx + 1) * 128],
                w_o[:, dh * 512 : (dh + 1) * 512],
                start=True,
                stop=True,
            )
            nc.vector.tensor_copy(og[:, dh * 512 : (dh + 1) * 512], pO[:])
            if dh == 1:
                nc.sync.dma_start(
                    out=out[
                        b * qlen + t_idx * 128 : b * qlen + (t_idx + 1) * 128, :
                    ],
                    in_=og[:],
                )

        open_groups = [0]  # psX accumulation groups not yet closed by bias

        def proj_closures(b):
            """(cost_ns, fn) DMA + projection + V-transpose closures for batch
            b. X-slice DMAs lead their projection groups by two slices so the
            PE never waits on an in-flight DMA inside the aux stream."""
            t = tiles[b]
            cls = []
            xis = {}
            pps = {}

            def dma_cl(i):
                def run():
                    xis[i] = emit_x_dma(b, i)

                return (0.0, run)

            def half_cl(i, w_s, f0, key):
                def run():
                    if (i, key) not in pps:
                        pps[(i, key)] = next_aux(f"pp{b}_{i}_{key}")
                        open_groups[0] += 1
                    emit_proj_half(b, i, xis[i], w_s, f0, pps[(i, key)])

                return (853.0, run)

            def bias_cl(i, b_s, key):
                def run():
                    emit_bias(b, i, b_s, t[key], pps.pop((i, key)))
                    open_groups[0] -= 1

                return (0.0, run)

            for i in range(min(3, tsl)):
                cls.append(dma_cl(i))
            for i in range(tsl):
                if i + 3 < tsl:
                    cls.append(dma_cl(i + 3))
                for w_s, b_s, key in (
                    (w_q, b_q, "qT"),
                    (w_k, b_k, "kT"),
                    (w_v, b_v, "vT"),
                ):
                    cls.append(half_cl(i, w_s, 0, key))
                    cls.append(half_cl(i, w_s, n_f // 2, key))
                    cls.append(bias_cl(i, b_s, key))
            cls.append((0.0, lambda: emit_vh_ones(b)))
            for j in range(jt):
                cls.append((53.0, lambda j=j: emit_vtrans(b, j)))
            return cls

        def outproj_closures(vb, ic):
            cls = []
            for t_idx in range(ic * 4, (ic + 1) * 4):
                og = [None]

                def mk(dh, t_idx=t_idx, og=og):
                    def run():
                        if og[0] is None:
                            og[0] = opool.tile(
                                [128, 1024], bf16, tag="og",
                                name=f"og{vb}_{t_idx}",
                            )
                        emit_outproj_half(vb, t_idx, dh, og[0])

                    return (427.0, run)

                cls.append(mk(0))
                cls.append(mk(1))
            return cls

        aux_q = deque()

        def pump_budget(budget_ns):
            spent = 0.0
            while aux_q and spent < budget_ns:
                cost, fn = aux_q.popleft()
                fn()
                spent += cost

        def drain_open_groups():
            # The inline ctx-transposes rotate the same psX slots as the
            # projection accumulators; entering them with a started-but-
            # unclosed projection group would deadlock the in-order PE
            # stream. Pump until every open group's bias has been emitted.
            while aux_q and open_groups[0] > 0:
                cost, fn = aux_q.popleft()
                fn()

        # ---- startup: batch 0 front matter emitted directly ----
        alloc_tiles(0)
        cls0 = proj_closures(0)
        for _, cl in cls0[:2]:  # first two X-slice DMAs
            cl()
        emit_late_consts()
        for _, cl in cls0[2:]:
            cl()

        # Attention i-chunks are 512 wide; the score PSUM tile holds both
        # heads side by side ([A | B]) so one exp op covers both and the two
        # K=64 score matmuls land in disjoint PE row groups (concurrent).
        AUX_BUDGET_NS = 450.0
        for vb in range(bs):
            b = vb
            t = tiles[vb]
            qT, kT = t["qT"], t["kT"]
            vh = t["vh"]
            # stage next batch's front matter into the aux stream
            if vb + 1 < bs:
                alloc_tiles(vb + 1)
                aux_q.extend(proj_closures(vb + 1))

            for ic in range(n_ica):
                isl = slice(ic * 512, (ic + 1) * 512)
                pC = [
                    psC.tile([128, 4 * 65], f32, tag=tg, name=f"p{tg}{vb}_{ic}")
                    for tg in ("pca", "pcb")
                ]

                # Software-pipelined: PV for j-LAG is issued after scores/exp
                # for j, so the in-order PE stream never waits on the exp of
                # the current iteration before reaching the next scores.
                def emit_pv(jj, exx):
                    for h in range(2):
                        for it in range(4):
                            nc.tensor.matmul(
                                pC[h][:, it * 65 : (it + 1) * 65],
                                exx[:, h * 512 + it * 128 : h * 512 + (it + 1) * 128],
                                vh[h][:, jj * 65 : (jj + 1) * 65],
                                start=(jj == 0),
                                stop=(jj == jt - 1),
                            )

                pend = deque()  # (j, ex) whose PV is not yet emitted
                for j in range(jt):
                    pS = psS.tile([128, 1024], f32, tag="ps", name=f"pS{vb}_{ic}_{j}")
                    for h in range(2):
                        hp = slice(64 * h, 64 * h + 64)
                        nc.tensor.matmul(
                            pS[:, h * 512 : (h + 1) * 512],
                            kT[hp, j * 128 : (j + 1) * 128],
                            qT[hp, isl],
                            start=True,
                            stop=True,
                        )
                    ex = epool.tile([128, 1024], bf16, tag="e", name=f"e{vb}_{ic}_{j}")
                    nc.scalar.activation(
                        ex[:],
                        pS[:],
                        EXP,
                        bias=mb_s[:, b * jt + j : b * jt + j + 1],
                        scale=1.0,
                    )
                    pend.append((j, ex))
                    if len(pend) > 4:
                        emit_pv(*pend.popleft())
                    pump_budget(AUX_BUDGET_NS)
                while pend:
                    emit_pv(*pend.popleft())

                drain_open_groups()
                # normalize: ctx[i, d] = pC[i, d] / pC[i, 64] (ones column)
                cxt = cxp.tile([128, 512], bf16, tag="cx", name=f"cx{vb}_{ic}")
                for h in range(2):
                    rr = rrp.tile([128, 4], f32, tag=f"rr{h}", name=f"rr{h}_{vb}_{ic}")
                    nc.vector.reciprocal(
                        rr[:].rearrange("p (a o) -> p a o", o=1),
                        pC[h][:].rearrange("p (it c) -> p it c", c=65)[:, :, 64:65],
                    )
                    for it in range(4):
                        nc.vector.tensor_scalar_mul(
                            cxt[:, it * 128 + h * 64 : it * 128 + h * 64 + 64],
                            pC[h][:, it * 65 : it * 65 + 64],
                            rr[:, it : it + 1],
                        )
                # transpose ctx back to [d, t] for the out-projection
                for it in range(4):
                    ptc = next_aux_bf(f"ptc{vb}_{ic}_{it}")
                    nc.tensor.transpose(
                        ptc[:, 0:128], cxt[:, it * 128 : (it + 1) * 128], ident[:]
                    )
                    nc.vector.tensor_copy(
                        t["ctxT"][:, (ic * 4 + it) * 128 : (ic * 4 + it + 1) * 128],
                        ptc[:, 0:128],
                    )
                # ctx columns for this i-chunk are complete: queue out-proj
                aux_q.extend(outproj_closures(vb, ic))

        # drain remaining aux work (at least the last i-chunk's out-proj)
        pump_budget(float("inf"))

    nc.compile()
    return nc


_NC_CACHE = {}


def _get_nc(bs=BS, qlen=QLEN):
    key = (bs, qlen)
    if key not in _NC_CACHE:
        _NC_CACHE[key] = build_nc(bs, qlen)
    return _NC_CACHE[key]


def make_in_maps(hidden_states, attention_mask, Wq, bq, Wk, bk, Wv, bv, Wo, bo):
    """Host-side sharding: per-core input dicts."""
    import ml_dtypes

    bs, qlen, dim = hidden_states.shape
    x = np.ascontiguousarray(
        hidden_states.reshape(bs * qlen, dim).T, dtype=np.float32
    )
    scale = 1.0 / np.sqrt(np.float32(DH))
    jt = qlen // 128
    maskbias = np.where(attention_mask == 0, np.float32(NEG_BIAS), np.float32(0.0))
    # mb[p, b*jt + j] = maskbias[b, j*128 + p]
    mb = np.ascontiguousarray(
        maskbias.reshape(bs, jt, 128).transpose(2, 0, 1).reshape(128, bs * jt),
        dtype=np.float32,
    )
    in_maps = []
    for c in range(NCORES):
        cs = slice(c * CPD, (c + 1) * CPD)
        in_maps.append(
            {
                "xt": x,
                "wq": np.ascontiguousarray((Wq[cs] * scale).T, dtype=np.float32),
                "wk": np.ascontiguousarray(Wk[cs].T, dtype=np.float32),
                "wv": np.ascontiguousarray(Wv[cs].T, dtype=np.float32),
                "wo": np.ascontiguousarray(Wo[:, cs].T).astype(ml_dtypes.bfloat16),
                "bq": np.ascontiguousarray(
                    (bq[cs] * scale)[:, None], dtype=np.float32
                ),
                "bk": np.ascontiguousarray(bk[cs][:, None], dtype=np.float32),
                "bv": np.ascontiguousarray(bv[cs][:, None], dtype=np.float32),
                "mb": mb,
            }
        )
    return in_maps


def kernel(hidden_states, attention_mask, Wq, bq, Wk, bk, Wv, bv, Wo, bo):
    from concourse.bass_utils import run_bass_kernel_spmd

    hidden_states = np.asarray(hidden_states, dtype=np.float32)
    attention_mask = np.asarray(attention_mask)
    Wq, bq = np.asarray(Wq, np.float32), np.asarray(bq, np.float32)
    Wk, bk = np.asarray(Wk, np.float32), np.asarray(bk, np.float32)
    Wv, bv = np.asarray(Wv, np.float32), np.asarray(bv, np.float32)
    Wo, bo = np.asarray(Wo, np.float32), np.asarray(bo, np.float32)

    bs, qlen, dim = hidden_states.shape
    nc = _get_nc(bs, qlen)
    in_maps = make_in_maps(
        hidden_states, attention_mask, Wq, bq, Wk, bk, Wv, bv, Wo, bo
    )
    res = run_bass_kernel_spmd(nc, in_maps, list(range(NCORES)))
    acc = res.results[0]["out"].astype(np.float32)
    for c in range(1, NCORES):
        acc = acc + res.results[c]["out"].astype(np.float32)
    acc = acc + bo[None, :]
    return acc.astype(np.float32).reshape(bs, qlen, dim)


# revision 12
# speedup vs baseline: 1.0769x; 1.0333x over previous
"""Multi-head attention TRN2 kernel, 8-core tensor-parallel (2 heads/core).

Strategy (per core c, head-slice cs = 128c:128c+128 of the projection dim):
  - Host passes X^T [1024, 8192] plus per-core weight slices, pre-transposed
    so every matmul operand lands in SBUF in its natural layout.
  - Projections compute Q^T/K^T [128, qlen] f32r and V^T [128, qlen] bf16
    per batch via PE accumulation over 8 f-tiles; bias added on DVE.
  - V^T is PE-transposed (bf16 identity, 1 cyc/row) into per-head V tiles
    vh[h] [j, 65] per j-tile: 64 head dims plus a ones column so the PV
    matmul also emits the softmax denominator for free.
  - Scores are computed transposed (S^T = K^T.T @ Q^T tiles, j on
    partitions); softmax skips max-subtraction (scores are O(6) for this
    problem's distribution so exp cannot overflow); the attention mask is
    folded into the exp activation as a per-partition bias (0 or -1e30).
  - PV runs in the narrow orientation: ctx[i, d] (i on partitions) with
    bf16 operands, N=65 per matmul instead of N=512 — half the PE cycles
    of the transposed orientation. ctx is normalized on DVE (recip of the
    ones-column sums), PE-transposed back to [d, t] for the out-proj.
  - Out-proj in bf16 (lhsT = ctx^T, rhs = Wo slice), partials written
    bf16 and summed on host.
  - Aux work (next batch's X-DMA/projections/V-transposes, previous
    i-chunks' out-projections) is interleaved into the ACT(exp)-paced
    attention loop with a PE-cycle budget per j-iteration so the PE never
    idles (idle resets the p-state ramp) and never outruns the 2-deep
    score-PSUM ring.
"""

import sys
from collections import deque

sys.path.insert(0, "/opt/trn_rl_repo")

import numpy as np

BS, QLEN, DIM, NH = 4, 2048, 1024, 16
DH = DIM // NH  # 64
NCORES = 8
CPD = DIM // NCORES  # 128 projection dims per core = 2 heads
T_FULL = BS * QLEN
NEG_BIAS = -1.0e30


def build_nc(bs=BS, qlen=QLEN):
    """Build + compile the per-core Bass program (same program on all cores)."""
    import concourse.bass as bass_mod
    import concourse.mybir as mybir
    import concourse.tile as tile
    from concourse import bacc
    from concourse import masks
    from contextlib import ExitStack

    f32 = mybir.dt.float32
    f32r = mybir.dt.float32r
    bf16 = mybir.dt.bfloat16
    EXP = mybir.ActivationFunctionType.Exp

    assert qlen % 512 == 0
    t_total = bs * qlen
    n_f = DIM // 128  # 8 f-tiles in the contraction over DIM
    jt = qlen // 128  # j-tiles (k-positions) per batch
    tsl = qlen // 512  # 512-slices per batch for projections
    n_ica = qlen // 512  # i-chunks per batch

    nc = bacc.Bacc()
    xt = nc.declare_dram_parameter("xt", [DIM, t_total], f32r, isOutput=False)
    wq = nc.declare_dram_parameter("wq", [DIM, CPD], f32r, isOutput=False)
    wk = nc.declare_dram_parameter("wk", [DIM, CPD], f32r, isOutput=False)
    wv = nc.declare_dram_parameter("wv", [DIM, CPD], f32r, isOutput=False)
    wo = nc.declare_dram_parameter("wo", [CPD, DIM], bf16, isOutput=False)
    bq = nc.declare_dram_parameter("bq", [CPD, 1], f32, isOutput=False)
    bk = nc.declare_dram_parameter("bk", [CPD, 1], f32, isOutput=False)
    bv = nc.declare_dram_parameter("bv", [CPD, 1], f32, isOutput=False)
    mb = nc.declare_dram_parameter("mb", [128, bs * jt], f32, isOutput=False)
    out = nc.declare_dram_parameter("out", [t_total, DIM], bf16, isOutput=True)

    xt_r = xt.rearrange("(n p) t -> n p t", p=128)

    with ExitStack() as ctx:
        tc = ctx.enter_context(tile.TileContext(nc))
        wpool = ctx.enter_context(tc.tile_pool(name="wpool", bufs=1))
        xpool = ctx.enter_context(tc.tile_pool(name="xpool", bufs=4))
        qkp = ctx.enter_context(tc.tile_pool(name="qkp", bufs=2))
        vtp = ctx.enter_context(tc.tile_pool(name="vtp", bufs=1))
        vhp = ctx.enter_context(tc.tile_pool(name="vhp", bufs=2))
        epool = ctx.enter_context(tc.tile_pool(name="epool", bufs=6))
        cxp = ctx.enter_context(tc.tile_pool(name="cxp", bufs=2))
        rrp = ctx.enter_context(tc.tile_pool(name="rrp", bufs=2))
        ctp = ctx.enter_context(tc.tile_pool(name="ctp", bufs=2))
        opool = ctx.enter_context(tc.tile_pool(name="opool", bufs=2))
        psS = ctx.enter_context(tc.tile_pool(name="psS", bufs=2, space="PSUM"))
        psC = ctx.enter_context(tc.tile_pool(name="psC", bufs=1, space="PSUM"))
        psX = ctx.enter_context(tc.tile_pool(name="psX", bufs=1, space="PSUM"))

        # ---- persistent weights / constants ----
        w_q = wpool.tile([128, n_f, CPD], f32r, tag="w_q")
        w_k = wpool.tile([128, n_f, CPD], f32r, tag="w_k")
        w_v = wpool.tile([128, n_f, CPD], f32r, tag="w_v")
        w_o = wpool.tile([128, DIM], bf16, tag="w_o")
        b_q = wpool.tile([128, 1], f32, tag="b_q")
        b_k = wpool.tile([128, 1], f32, tag="b_k")
        b_v = wpool.tile([128, 1], f32, tag="b_v")
        mb_s = wpool.tile([128, bs * jt], f32, tag="mb")
        ident = wpool.tile([128, 128], bf16, tag="ident")

        # Order matters: these share the HWDGE queue with the first X-slice
        # DMA, so only what the first projection group needs goes first.
        nc.sync.dma_start(out=w_q[:], in_=wq.rearrange("(n p) m -> p n m", p=128))
        nc.sync.dma_start(out=b_q[:], in_=bq[:])

        def emit_late_consts():
            nc.sync.dma_start(
                out=w_k[:], in_=wk.rearrange("(n p) m -> p n m", p=128)
            )
            nc.sync.dma_start(out=b_k[:], in_=bk[:])
            nc.sync.dma_start(
                out=w_v[:], in_=wv.rearrange("(n p) m -> p n m", p=128)
            )
            nc.sync.dma_start(out=b_v[:], in_=bv[:])
            nc.sync.dma_start(out=w_o[:], in_=wo[:])
            nc.sync.dma_start(out=mb_s[:], in_=mb[:])
        masks.make_identity(nc, ident[:])

        # per-batch tile sets, allocated one batch ahead
        tiles = {}

        def alloc_tiles(b):
            t = {}
            t["qT"] = qkp.tile([128, qlen], f32r, tag="qT", name=f"qT{b}")
            t["kT"] = qkp.tile([128, qlen], f32r, tag="kT", name=f"kT{b}")
            t["vT"] = vtp.tile([128, qlen], bf16, tag="vT", name=f"vT{b}")
            t["vh"] = [
                vhp.tile([128, jt * 65], bf16, tag=f"vh{h}", name=f"vh{h}_{b}")
                for h in range(2)
            ]
            t["ctxT"] = ctp.tile([128, qlen], bf16, tag="ctxT", name=f"ctxT{b}")
            tiles[b] = t
            return t

        def emit_x_dma(b, i):
            xi = xpool.tile([128, n_f, 512], f32r, tag="x", name=f"x{b}_{i}")
            src = xt_r[:, :, b * qlen + i * 512 : b * qlen + (i + 1) * 512]
            # The very first X slice rides the (startup-idle) ACT DGE queue so
            # it lands in parallel with the weight DMAs on the SP queue.
            eng = nc.scalar if (b == 0 and i == 0) else nc.sync
            eng.dma_start(out=xi[:], in_=src.rearrange("f p t -> p f t"))
            return xi

        aux_n = [0]

        def next_aux(name):
            tg = ("auxA", "auxB")[aux_n[0] % 2]
            aux_n[0] += 1
            return psX.tile([128, 512], f32, tag=tg, name=f"{name}_{aux_n[0]}")

        def next_aux_bf(name):
            # Same psX slot rotation, viewed as bf16 (same byte size).
            tg = ("auxA", "auxB")[aux_n[0] % 2]
            aux_n[0] += 1
            return psX.tile([128, 1024], bf16, tag=tg, name=f"{name}_{aux_n[0]}")

        def emit_proj_step(b, i, xi, w_s, f, pp):
            nc.tensor.matmul(
                pp[:],
                w_s[:, f, :],
                xi[:, f, :],
                start=(f == 0),
                stop=(f == n_f - 1),
            )

        def emit_bias(b, i, b_s, dst, pp):
            nc.vector.tensor_scalar_add(
                dst[:, i * 512 : (i + 1) * 512], pp[:], b_s[:]
            )

        def emit_vtrans(b, j):
            t = tiles[b]
            pt = next_aux_bf(f"pt{b}_{j}")
            nc.tensor.transpose(
                pt[:, 0:128], t["vT"][:, j * 128 : (j + 1) * 128], ident[:]
            )
            for h in range(2):
                nc.vector.tensor_copy(
                    t["vh"][h][:, j * 65 : j * 65 + 64],
                    pt[:, h * 64 : h * 64 + 64],
                )

        def emit_vh_ones(b):
            for h in range(2):
                vh_c = tiles[b]["vh"][h][:].rearrange(
                    "p (j c) -> p j c", c=65
                )[:, :, 64:65]
                nc.vector.memset(vh_c, 1.0)

        def emit_outproj_half(vb, t_idx, dh, og):
            b = vb % bs
            t = tiles[vb]
            pO = next_aux(f"pO{vb}_{t_idx}_{dh}")
            nc.tensor.matmul(
                pO[:],
                t["ctxT"][:, t_idx * 128 : (t_idx + 1) * 128],
                w_o[:, dh * 512 : (dh + 1) * 512],
                start=True,
                stop=True,
            )
            nc.vector.tensor_copy(og[:, dh * 512 : (dh + 1) * 512], pO[:])
            if dh == 1:
                nc.sync.dma_start(
                    out=out[
                        b * qlen + t_idx * 128 : b * qlen + (t_idx + 1) * 128, :
                    ],
                    in_=og[:],
                )

        open_groups = [0]  # psX accumulation groups not yet closed by bias

        def proj_closures(b):
            """(cost_ns, fn) DMA + projection + V-transpose closures for batch
            b. X-slice DMAs lead their projection groups by two slices so the
            PE never waits on an in-flight DMA inside the aux stream."""
            t = tiles[b]
            cls = []
            xis = {}
            pps = {}

            def dma_cl(i):
                def run():
                    xis[i] = emit_x_dma(b, i)

                return (0.0, run)

            def step_cl(i, w_s, f, key):
                def run():
                    if (i, key) not in pps:
                        pps[(i, key)] = next_aux(f"pp{b}_{i}_{key}")
                        open_groups[0] += 1
                    emit_proj_step(b, i, xis[i], w_s, f, pps[(i, key)])

                return (213.0, run)

            def bias_cl(i, b_s, key):
                def run():
                    emit_bias(b, i, b_s, t[key], pps.pop((i, key)))
                    open_groups[0] -= 1

                return (0.0, run)

            for i in range(min(3, tsl)):
                cls.append(dma_cl(i))
            for i in range(tsl):
                if i + 3 < tsl:
                    cls.append(dma_cl(i + 3))
                for w_s, b_s, key in (
                    (w_q, b_q, "qT"),
                    (w_k, b_k, "kT"),
                    (w_v, b_v, "vT"),
                ):
                    for f in range(n_f):
                        cls.append(step_cl(i, w_s, f, key))
                    cls.append(bias_cl(i, b_s, key))
            cls.append((0.0, lambda: emit_vh_ones(b)))
            for j in range(jt):
                cls.append((53.0, lambda j=j: emit_vtrans(b, j)))
            return cls

        def outproj_closures(vb, ic):
            cls = []
            for t_idx in range(ic * 4, (ic + 1) * 4):
                og = [None]

                def mk(dh, t_idx=t_idx, og=og):
                    def run():
                        if og[0] is None:
                            og[0] = opool.tile(
                                [128, 1024], bf16, tag="og",
                                name=f"og{vb}_{t_idx}",
                            )
                        emit_outproj_half(vb, t_idx, dh, og[0])

                    return (213.0, run)

                cls.append(mk(0))
                cls.append(mk(1))
            return cls

        aux_q = deque()
        tokens = [0.0]

        def pump_tokens(rate_ns):
            # Token bucket: average aux PE-time per j-iteration equals
            # rate_ns, with jitter bounded by one quantum, so iteration
            # times stay smooth against the fixed exp cadence.
            tokens[0] = min(tokens[0] + rate_ns, 4.0 * rate_ns)
            while aux_q and tokens[0] > 0.0:
                cost, fn = aux_q.popleft()
                fn()
                tokens[0] -= cost

        def drain_open_groups():
            # The inline ctx-transposes rotate the same psX slots as the
            # projection accumulators; entering them with a started-but-
            # unclosed projection group would deadlock the in-order PE
            # stream. Pump until every open group's bias has been emitted.
            while aux_q and open_groups[0] > 0:
                cost, fn = aux_q.popleft()
                fn()

        # ---- startup: batch 0 front matter emitted directly ----
        alloc_tiles(0)
        cls0 = proj_closures(0)
        for _, cl in cls0[:2]:  # first two X-slice DMAs
            cl()
        emit_late_consts()
        for _, cl in cls0[2:]:
            cl()

        # Attention i-chunks are 512 wide; the score PSUM tile holds both
        # heads side by side ([A | B]) so one exp op covers both and the two
        # K=64 score matmuls land in disjoint PE row groups (concurrent).
        AUX_RATE_NS = 550.0
        for vb in range(bs):
            b = vb
            t = tiles[vb]
            qT, kT = t["qT"], t["kT"]
            vh = t["vh"]
            # stage next batch's front matter into the aux stream
            if vb + 1 < bs:
                alloc_tiles(vb + 1)
                aux_q.extend(proj_closures(vb + 1))

            for ic in range(n_ica):
                isl = slice(ic * 512, (ic + 1) * 512)
                pC = [
                    psC.tile([128, 4 * 65], f32, tag=tg, name=f"p{tg}{vb}_{ic}")
                    for tg in ("pca", "pcb")
                ]

                # Software-pipelined: PV for j-LAG is issued after scores/exp
                # for j, so the in-order PE stream never waits on the exp of
                # the current iteration before reaching the next scores.
                def emit_pv(jj, exx):
                    for h in range(2):
                        for it in range(4):
                            nc.tensor.matmul(
                                pC[h][:, it * 65 : (it + 1) * 65],
                                exx[:, h * 512 + it * 128 : h * 512 + (it + 1) * 128],
                                vh[h][:, jj * 65 : (jj + 1) * 65],
                                start=(jj == 0),
                                stop=(jj == jt - 1),
                            )

                pend = deque()  # (j, ex) whose PV is not yet emitted
                for j in range(jt):
                    pS = psS.tile([128, 1024], f32, tag="ps", name=f"pS{vb}_{ic}_{j}")
                    for h in range(2):
                        hp = slice(64 * h, 64 * h + 64)
                        nc.tensor.matmul(
                            pS[:, h * 512 : (h + 1) * 512],
                            kT[hp, j * 128 : (j + 1) * 128],
                            qT[hp, isl],
                            start=True,
                            stop=True,
                        )
                    ex = epool.tile([128, 1024], bf16, tag="e", name=f"e{vb}_{ic}_{j}")
                    nc.scalar.activation(
                        ex[:],
                        pS[:],
                        EXP,
                        bias=mb_s[:, b * jt + j : b * jt + j + 1],
                        scale=1.0,
                    )
                    pend.append((j, ex))
                    if len(pend) > 4:
                        emit_pv(*pend.popleft())
                    pump_tokens(AUX_RATE_NS)
                while pend:
                    emit_pv(*pend.popleft())

                drain_open_groups()
                # normalize: ctx[i, d] = pC[i, d] / pC[i, 64] (ones column)
                cxt = cxp.tile([128, 512], bf16, tag="cx", name=f"cx{vb}_{ic}")
                for h in range(2):
                    rr = rrp.tile([128, 4], f32, tag=f"rr{h}", name=f"rr{h}_{vb}_{ic}")
                    nc.vector.reciprocal(
                        rr[:].rearrange("p (a o) -> p a o", o=1),
                        pC[h][:].rearrange("p (it c) -> p it c", c=65)[:, :, 64:65],
                    )
                    for it in range(4):
                        nc.vector.tensor_scalar_mul(
                            cxt[:, it * 128 + h * 64 : it * 128 + h * 64 + 64],
                            pC[h][:, it * 65 : it * 65 + 64],
                            rr[:, it : it + 1],
                        )
                # transpose ctx back to [d, t] for the out-projection
                for it in range(4):
                    ptc = next_aux_bf(f"ptc{vb}_{ic}_{it}")
                    nc.tensor.transpose(
                        ptc[:, 0:128], cxt[:, it * 128 : (it + 1) * 128], ident[:]
                    )
                    nc.vector.tensor_copy(
                        t["ctxT"][:, (ic * 4 + it) * 128 : (ic * 4 + it + 1) * 128],
                        ptc[:, 0:128],
                    )
                # ctx columns for this i-chunk are complete: queue out-proj
                aux_q.extend(outproj_closures(vb, ic))

        # drain remaining aux work (at least the last i-chunk's out-proj)
        tokens[0] = float("inf")
        pump_tokens(0.0)

    nc.compile()
    return nc


_NC_CACHE = {}


def _get_nc(bs=BS, qlen=QLEN):
    key = (bs, qlen)
    if key not in _NC_CACHE:
        _NC_CACHE[key] = build_nc(bs, qlen)
    return _NC_CACHE[key]


def make_in_maps(hidden_states, attention_mask, Wq, bq, Wk, bk, Wv, bv, Wo, bo):
    """Host-side sharding: per-core input dicts."""
    import ml_dtypes

    bs, qlen, dim = hidden_states.shape
    x = np.ascontiguousarray(
        hidden_states.reshape(bs * qlen, dim).T, dtype=np.float32
    )
    scale = 1.0 / np.sqrt(np.float32(DH))
    jt = qlen // 128
    maskbias = np.where(attention_mask == 0, np.float32(NEG_BIAS), np.float32(0.0))
    # mb[p, b*jt + j] = maskbias[b, j*128 + p]
    mb = np.ascontiguousarray(
        maskbias.reshape(bs, jt, 128).transpose(2, 0, 1).reshape(128, bs * jt),
        dtype=np.float32,
    )
    in_maps = []
    for c in range(NCORES):
        cs = slice(c * CPD, (c + 1) * CPD)
        in_maps.append(
            {
                "xt": x,
                "wq": np.ascontiguousarray((Wq[cs] * scale).T, dtype=np.float32),
                "wk": np.ascontiguousarray(Wk[cs].T, dtype=np.float32),
                "wv": np.ascontiguousarray(Wv[cs].T, dtype=np.float32),
                "wo": np.ascontiguousarray(Wo[:, cs].T).astype(ml_dtypes.bfloat16),
                "bq": np.ascontiguousarray(
                    (bq[cs] * scale)[:, None], dtype=np.float32
                ),
                "bk": np.ascontiguousarray(bk[cs][:, None], dtype=np.float32),
                "bv": np.ascontiguousarray(bv[cs][:, None], dtype=np.float32),
                "mb": mb,
            }
        )
    return in_maps


def kernel(hidden_states, attention_mask, Wq, bq, Wk, bk, Wv, bv, Wo, bo):
    from concourse.bass_utils import run_bass_kernel_spmd

    hidden_states = np.asarray(hidden_states, dtype=np.float32)
    attention_mask = np.asarray(attention_mask)
    Wq, bq = np.asarray(Wq, np.float32), np.asarray(bq, np.float32)
    Wk, bk = np.asarray(Wk, np.float32), np.asarray(bk, np.float32)
    Wv, bv = np.asarray(Wv, np.float32), np.asarray(bv, np.float32)
    Wo, bo = np.asarray(Wo, np.float32), np.asarray(bo, np.float32)

    bs, qlen, dim = hidden_states.shape
    nc = _get_nc(bs, qlen)
    in_maps = make_in_maps(
        hidden_states, attention_mask, Wq, bq, Wk, bk, Wv, bv, Wo, bo
    )
    res = run_bass_kernel_spmd(nc, in_maps, list(range(NCORES)))
    acc = res.results[0]["out"].astype(np.float32)
    for c in range(1, NCORES):
        acc = acc + res.results[c]["out"].astype(np.float32)
    acc = acc + bo[None, :]
    return acc.astype(np.float32).reshape(bs, qlen, dim)
